# revision 30
# baseline (speedup 1.0000x reference)
"""Bass/Trainium2 kernel for nn_Attention (additive attention, dense_transformer).

Strategy: pure data-parallel over batch N=16 across 8 NeuronCores (2 batches
per core), no collectives. Per core:
  PE   fc_create      qh_sb[e, b, q, h] (bf16, bias fused in ACT copy)
  DVE  broadcast-add  arg[e, qh, v] = qh_sb[e, qh] + cT[e, v]        (bf16)
  ACT  tanh           t = tanh(arg)                                   (bf16)
  PE   logits         128 wide matmuls: lhsT = w_logit [e,1] stationary,
                      rhs = t[:, 4qh-group, :] (512 cols) -> pl3[g, (h, v)]
  DVE  masking        u = pl3 * (mask/T)_rep + (b_logit*mask/T - 99999*(1-mask))_rep
  ACT  exp + accum    exp2[g, h, v] bf16, denominators via accum_out
  DVE  normalize      probs = exp2 * (1/den)[g, h]   (pre-normalized!)
  PE   transpose      probs [g, h, v] -> probsT [v, b, qh]
  PE   heads^T        pheT[e, qh] = memM[v, e].T @ probsT   (mem host-premasked)
  DVE  leaky_relu     HeT = max(pheT, 0.01*pheT)  (already in fc_reduce layout)
  PE   fc_reduce      out[q, o] (b_reduce added host-side)

Walrus supports only ONE sync-wait per compute instruction micro-op; Tile can
emit several. `_split_waits` hoists extra waits into standalone NoOps right
before the instruction. PSUM tiles are persistent with disjoint slices per
use (PSUM slot reuse makes Tile emit same-engine WAW waits).
"""

import numpy as np
import ml_dtypes

try:
    import concourse.bass as bass
except ImportError:
    import sys
    sys.path.insert(0, "/opt/trn_rl_repo")
    import concourse.bass as bass
import concourse.mybir as mybir
import concourse.tile as tile
from concourse.bass_utils import run_bass_kernel_spmd

N, nQ, nV, nH, nE = 16, 64, 128, 4, 128
NCORES = 8
B = N // NCORES      # batches per core
QH = nQ * nH         # 256
BLK = 64             # qh per work block
NBLK = QH // BLK     # 4
QBLK = BLK // nH     # q's per block (16)
NG = B * nQ          # logits groups per core (one group = 4 qh = one q) = 128
F32 = mybir.dt.float32
BF16 = mybir.dt.bfloat16
AF = mybir.ActivationFunctionType
BFNP = ml_dtypes.bfloat16

_SPLIT_ENGINES = {
    mybir.EngineType.PE,
    mybir.EngineType.DVE,
    mybir.EngineType.Activation,
    mybir.EngineType.Pool,
    mybir.EngineType.SP,
}
_NO_SPLIT_OPS = {"TriggeredCopy", "EventSemaphore", "NoOp",
                 "UnconditionalBranch", "RegisterMove", "Halt", "BranchHint"}


def _split_waits(nc):
    nid = 0
    for f in nc.m.functions:
        for blk in f.blocks:
            out = []
            for inst in blk.instructions:
                si = inst.sync_info
                if (si is not None and len(si.on_wait) > 1
                        and inst.engine in _SPLIT_ENGINES
                        and str(inst.opcode) not in _NO_SPLIT_OPS):
                    waits = list(si.on_wait)
                    for w in waits[:-1]:
                        nid += 1
                        nop = mybir.InstNoOp(name=f"I-wsplit-{nid}",
                                             ins=[], outs=[])
                        nop.engine = inst.engine
                        nop.sync_info = mybir.SyncInfo(on_wait=[w],
                                                       on_update=[])
                        out.append(nop)
                    inst.sync_info = mybir.SyncInfo(
                        on_wait=[waits[-1]], on_update=list(si.on_update))
                out.append(inst)
            blk.instructions[:] = out


def _build_nc():
    nc = bass.Bass()
    qT = nc.declare_dram_parameter("qT", [B, nE, nQ], BF16, isOutput=False)
    cT = nc.declare_dram_parameter("cT", [B, nE, nV], F32, isOutput=False)
    memM = nc.declare_dram_parameter("memM", [B, nV, nE], BF16, isOutput=False)
    WcT = nc.declare_dram_parameter("WcT", [nE, nH * nE], BF16, isOutput=False)
    WrT = nc.declare_dram_parameter("WrT", [nE, nH, nE], BF16, isOutput=False)
    bC = nc.declare_dram_parameter("bC", [nE, nH], F32, isOutput=False)
    wI = nc.declare_dram_parameter("wI", [nE, 32, 32], BF16, isOutput=False)
    mbi = nc.declare_dram_parameter("mbi", [1, B, nH * nV], BF16, isOutput=False)
    outp = nc.declare_dram_parameter("out", [B, nQ, nE], F32, isOutput=True)

    with tile.TileContext(nc) as tc:
        with tc.tile_pool(name="singles", bufs=1) as singles, \
             tc.tile_pool(name="argp", bufs=2) as argp, \
             tc.tile_pool(name="tp", bufs=2) as tp, \
             tc.tile_pool(name="obp", bufs=2) as obp, \
             tc.tile_pool(name="psing", bufs=1, space="PSUM") as psing:

            # ---- persistent PSUM tiles (disjoint slices; 5 banks) ----
            pls = [psing.tile([32, nH, nV], F32, name=f"pl{i}", tag=f"pl{i}")
                   for i in range(4)]               # logits [g%32, h, v] x4
            pqc_all = psing.tile([nE, nH, B * nQ], F32)  # fc_create out
            phe = psing.tile([nE, B, QH], F32)          # heads^T, 1 bank
            po_all = psing.tile([B * nQ, nE], F32)      # final out

            # ---- constants / persistent SBUF tiles ----
            WcT_sb = singles.tile([nE, nH * nE], BF16)
            nc.sync.dma_start(out=WcT_sb, in_=WcT[:, :])
            bC_sb = singles.tile([nE, nH], F32)
            nc.sync.dma_start(out=bC_sb, in_=bC[:, :])
            wI_sb = singles.tile([nE, 32, 32], BF16)
            nc.gpsimd.dma_start(out=wI_sb, in_=wI[:, :, :])
            WrT_sb = singles.tile([nE, nH, nE], BF16)
            nc.gpsimd.dma_start(out=WrT_sb, in_=WrT[:, :, :])
            mbi_sb = singles.tile([1, B, nH * nV], BF16)
            nc.sync.dma_start(out=mbi_sb, in_=mbi[:, :, :])
            ones32 = singles.tile([1, 32], BF16)
            nc.vector.memset(ones32, 1.0)
            cT_sb = singles.tile([nE, B, nV], F32)
            memM_sb = singles.tile([nV, B, nE], BF16)
            qTq_sb = singles.tile([nE, B, nQ], BF16)
            for b in range(B):
                nc.sync.dma_start(out=qTq_sb[:, b, :], in_=qT[b])
                nc.sync.dma_start(out=cT_sb[:, b, :], in_=cT[b])
                nc.gpsimd.dma_start(out=memM_sb[:, b, :], in_=memM[b])
            qh_sb = singles.tile([nE, B, nQ, nH], F32)    # fc_create out ^T
            exp_sb = singles.tile([NG, nH, nV], BF16)     # exp(masked logits)
            den_sb = singles.tile([NG, nH], F32)          # softmax denominators
            rec_sb = singles.tile([NG, nH], F32)          # 1/den
            probs_sb = singles.tile([NG, nH, nV], BF16)   # normalized probs
            HeT_sb = singles.tile([nE, B, nQ, nH], BF16)  # leaky heads^T
            hm_sb = singles.tile([nE, B, QH], F32)        # 0.01*heads scratch

            # ---- fc_createheads (batched over b: rhs = [k, B*nQ]) ----
            qTq_flat = qTq_sb[:, :, :].rearrange("k b q -> k (b q)")
            for h in range(nH):
                pqc = pqc_all[:, h, :]
                nc.tensor.matmul(pqc, WcT_sb[:, h * nE : (h + 1) * nE],
                                 qTq_flat, start=True, stop=True)
                nc.scalar.activation(out=qh_sb[:, :, :, h], in_=pqc,
                                     func=AF.Identity, bias=bC_sb[:, h : h + 1])

            # ---- main loop: broadcast-add, tanh, wide logits matmuls ----
            for b in range(B):
                for blk in range(NBLK):
                    arg = argp.tile([nE, BLK, nV], F32)
                    qsl = qh_sb[:, b, blk * QBLK : (blk + 1) * QBLK, :]
                    k = b * NBLK + blk
                    add_eng = nc.gpsimd if k in (3, 7) else nc.vector
                    add_eng.tensor_add(
                        arg,
                        qsl[:, :, :, None].broadcast_to([nE, QBLK, nH, nV]),
                        cT_sb[:, b, None, :].broadcast_to([nE, BLK, nV]),
                    )
                    t = tp.tile([nE, BLK, nV], BF16)
                    nc.scalar.activation(out=t, in_=arg, func=AF.Tanh)
                    for q4 in range(QBLK):
                        g = b * nQ + blk * QBLK + q4
                        i, r = g // 32, g % 32
                        if r == 0:
                            # inject the mask bias row: -99999*(1-m) etc.
                            nc.tensor.matmul(pls[i], ones32,
                                             mbi_sb[:, i // 2, :],
                                             start=True, stop=False)
                        nc.tensor.matmul(
                            pls[i], wI_sb[:, r, :],
                            t[:, q4 * nH : (q4 + 1) * nH, :],
                            start=False, stop=(r == 31))

            # ---- exp + denominators (straight from PSUM) ----
            for i in range(4):
                for h in range(nH):
                    nc.scalar.activation(
                        out=exp_sb[32 * i : 32 * (i + 1), h, :],
                        in_=pls[i][:, h, :], func=AF.Exp,
                        accum_out=den_sb[32 * i : 32 * (i + 1), h : h + 1])
            nc.vector.reciprocal(rec_sb, den_sb)
            for h in range(nH):
                nc.vector.tensor_scalar_mul(probs_sb[:, h, :],
                                            exp_sb[:, h, :],
                                            rec_sb[:, h : h + 1])

            # ---- transpose probs -> ptrT[v, h, g] via DMA xbar ----
            ptrT_sb = singles.tile([nV, nH, NG], BF16)
            for h in range(nH):
                nc.sync.dma_start_transpose(ptrT_sb[:, h, :], probs_sb[:, h, :])

            # ---- heads^T: phe[e, b, qh] = memM[v, e].T @ probsT ----
            for b in range(B):
                rhs = ptrT_sb[:, :, b * nQ : (b + 1) * nQ].rearrange(
                    "v h q -> v q h")
                nc.tensor.matmul(phe[:, b, :], memM_sb[:, b, :],
                                 rhs, start=True, stop=True)

            # ---- leaky_relu into fc_reduce layout ----
            nc.vector.tensor_scalar_mul(hm_sb, phe, 0.01)
            nc.vector.tensor_max(HeT_sb, phe, hm_sb)

            # ---- fc_reduceheads (batched over b; b_reduce added host-side) ----
            for h in range(nH):
                nc.tensor.matmul(
                    po_all, HeT_sb[:, :, :, h].rearrange("e b q -> e (b q)"),
                    WrT_sb[:, h, :], start=(h == 0), stop=(h == nH - 1))
            ob = obp.tile([B * nQ, nE], F32)
            nc.vector.tensor_copy(ob, po_all)
            nc.sync.dma_start(out=outp[:, :, :].rearrange("b q e -> (b q) e"), in_=ob)

    _split_waits(nc)
    return nc


_NC_CACHE = None


def _get_nc():
    global _NC_CACHE
    if _NC_CACHE is None:
        _NC_CACHE = _build_nc()
    return _NC_CACHE


def _prep_in_maps(inputs):
    query = np.asarray(inputs["query"], np.float32)
    context = np.asarray(inputs["context"], np.float32)
    memory = np.asarray(inputs["memory"], np.float32)
    mask = np.asarray(inputs["mask"], np.float32)
    W_create = np.asarray(inputs["W_create"], np.float32)
    b_create = np.asarray(inputs["b_create"], np.float32)
    w_logit = np.asarray(inputs["w_logit"], np.float32)
    b_logit = float(np.asarray(inputs["b_logit"], np.float32))
    W_reduce = np.asarray(inputs["W_reduce"], np.float32)

    WcT = np.ascontiguousarray(W_create.T.astype(BFNP))          # [k, he]
    WrT = np.ascontiguousarray(
        W_reduce.T.reshape(nH, nE, nE).transpose(1, 0, 2).astype(BFNP))
    bC = np.ascontiguousarray(b_create.reshape(nH, nE).T)        # [e, h]
    T = float(np.asarray(inputs["temperature"], np.float32))
    wI = np.zeros((nE, 32, 32), np.float32)
    wI[:, np.arange(32), np.arange(32)] = w_logit[:, None] / T
    wI = np.ascontiguousarray(wI.astype(BFNP))                   # (w/T) (x) I_32

    in_maps = []
    for i in range(NCORES):
        bs = slice(B * i, B * (i + 1))
        m = mask[bs]                                             # [B, nV]
        mbias = np.tile(b_logit * m / T - 30000.0 * (1.0 - m), (1, nH))
        memM = memory[bs] * m[:, :, None]                        # premasked
        in_maps.append({
            "qT": np.ascontiguousarray(
                query[bs].transpose(0, 2, 1).astype(BFNP)),
            "cT": np.ascontiguousarray(context[bs].transpose(0, 2, 1)),
            "memM": np.ascontiguousarray(memM.astype(BFNP)),
            "WcT": WcT, "WrT": WrT, "bC": bC, "wI": wI,
            "mbi": np.ascontiguousarray(mbias[None].astype(BFNP)),
        })
    return in_maps


def _run(inputs, trace=False, tmpdir=None):
    nc = _get_nc()
    in_maps = _prep_in_maps(inputs)
    res = run_bass_kernel_spmd(nc, in_maps, core_ids=list(range(NCORES)),
                               trace=trace, tmpdir=tmpdir)
    out = np.concatenate([res.results[i]["out"] for i in range(NCORES)], axis=0)
    out = out + np.asarray(inputs["b_reduce"], np.float32)[None, None, :]
    return np.ascontiguousarray(out.astype(np.float32)), res


def kernel(**inputs):
    out, _ = _run(inputs, trace=False)
    return out


# revision 32
# speedup vs baseline: 1.0258x; 1.0258x over previous
"""Bass/Trainium2 kernel for nn_Attention (additive attention, dense_transformer).

Strategy: pure data-parallel over batch N=16 across 8 NeuronCores (2 batches
per core), no collectives. Per core:
  PE   fc_create      qh_sb[e, b, q, h] (bf16, bias fused in ACT copy)
  DVE  broadcast-add  arg[e, qh, v] = qh_sb[e, qh] + cT[e, v]        (bf16)
  ACT  tanh           t = tanh(arg)                                   (bf16)
  PE   logits         128 wide matmuls: lhsT = w_logit [e,1] stationary,
                      rhs = t[:, 4qh-group, :] (512 cols) -> pl3[g, (h, v)]
  DVE  masking        u = pl3 * (mask/T)_rep + (b_logit*mask/T - 99999*(1-mask))_rep
  ACT  exp + accum    exp2[g, h, v] bf16, denominators via accum_out
  DVE  normalize      probs = exp2 * (1/den)[g, h]   (pre-normalized!)
  PE   transpose      probs [g, h, v] -> probsT [v, b, qh]
  PE   heads^T        pheT[e, qh] = memM[v, e].T @ probsT   (mem host-premasked)
  DVE  leaky_relu     HeT = max(pheT, 0.01*pheT)  (already in fc_reduce layout)
  PE   fc_reduce      out[q, o] (b_reduce added host-side)

Walrus supports only ONE sync-wait per compute instruction micro-op; Tile can
emit several. `_split_waits` hoists extra waits into standalone NoOps right
before the instruction. PSUM tiles are persistent with disjoint slices per
use (PSUM slot reuse makes Tile emit same-engine WAW waits).
"""

import numpy as np
import ml_dtypes

try:
    import concourse.bass as bass
except ImportError:
    import sys
    sys.path.insert(0, "/opt/trn_rl_repo")
    import concourse.bass as bass
import concourse.mybir as mybir
import concourse.tile as tile
from concourse.bass_utils import run_bass_kernel_spmd

N, nQ, nV, nH, nE = 16, 64, 128, 4, 128
NCORES = 8
B = N // NCORES      # batches per core
QH = nQ * nH         # 256
BLK = 64             # qh per work block
NBLK = QH // BLK     # 4
QBLK = BLK // nH     # q's per block (16)
NG = B * nQ          # logits groups per core (one group = 4 qh = one q) = 128
F32 = mybir.dt.float32
BF16 = mybir.dt.bfloat16
AF = mybir.ActivationFunctionType
BFNP = ml_dtypes.bfloat16

_SPLIT_ENGINES = {
    mybir.EngineType.PE,
    mybir.EngineType.DVE,
    mybir.EngineType.Activation,
    mybir.EngineType.Pool,
    mybir.EngineType.SP,
}
_NO_SPLIT_OPS = {"TriggeredCopy", "EventSemaphore", "NoOp",
                 "UnconditionalBranch", "RegisterMove", "Halt", "BranchHint"}


def _split_waits(nc):
    nid = 0
    for f in nc.m.functions:
        for blk in f.blocks:
            out = []
            for inst in blk.instructions:
                si = inst.sync_info
                if (si is not None and len(si.on_wait) > 1
                        and inst.engine in _SPLIT_ENGINES
                        and str(inst.opcode) not in _NO_SPLIT_OPS):
                    waits = list(si.on_wait)
                    for w in waits[:-1]:
                        nid += 1
                        nop = mybir.InstNoOp(name=f"I-wsplit-{nid}",
                                             ins=[], outs=[])
                        nop.engine = inst.engine
                        nop.sync_info = mybir.SyncInfo(on_wait=[w],
                                                       on_update=[])
                        out.append(nop)
                    inst.sync_info = mybir.SyncInfo(
                        on_wait=[waits[-1]], on_update=list(si.on_update))
                out.append(inst)
            blk.instructions[:] = out


def _build_nc():
    nc = bass.Bass()
    qT = nc.declare_dram_parameter("qT", [B, nE, nQ], BF16, isOutput=False)
    cT = nc.declare_dram_parameter("cT", [B, nE, nV], F32, isOutput=False)
    memM = nc.declare_dram_parameter("memM", [B, nV, nE], BF16, isOutput=False)
    WcT = nc.declare_dram_parameter("WcT", [nE, nH * nE], BF16, isOutput=False)
    WrT = nc.declare_dram_parameter("WrT", [nE, nH, nE], BF16, isOutput=False)
    bC = nc.declare_dram_parameter("bC", [nE, nH], F32, isOutput=False)
    wI = nc.declare_dram_parameter("wI", [nE, 32, 32], BF16, isOutput=False)
    mbi = nc.declare_dram_parameter("mbi", [1, B, nH * nV], BF16, isOutput=False)
    outp = nc.declare_dram_parameter("out", [B, nQ, nE], F32, isOutput=True)

    with tile.TileContext(nc) as tc:
        with tc.tile_pool(name="singles", bufs=1) as singles, \
             tc.tile_pool(name="argp", bufs=3) as argp, \
             tc.tile_pool(name="tp", bufs=3) as tp, \
             tc.tile_pool(name="obp", bufs=2) as obp, \
             tc.tile_pool(name="psing", bufs=1, space="PSUM") as psing:

            # ---- persistent PSUM tiles (disjoint slices; 5 banks) ----
            pls = [psing.tile([32, nH, nV], F32, name=f"pl{i}", tag=f"pl{i}")
                   for i in range(4)]               # logits [g%32, h, v] x4
            pqc_all = psing.tile([nE, nH, B * nQ], F32)  # fc_create out
            phe = psing.tile([nE, B, QH], F32)          # heads^T, 1 bank
            po_all = psing.tile([B * nQ, nE], F32)      # final out

            # ---- constants / persistent SBUF tiles ----
            WcT_sb = singles.tile([nE, nH * nE], BF16)
            nc.sync.dma_start(out=WcT_sb, in_=WcT[:, :])
            bC_sb = singles.tile([nE, nH], F32)
            nc.sync.dma_start(out=bC_sb, in_=bC[:, :])
            wI_sb = singles.tile([nE, 32, 32], BF16)
            nc.gpsimd.dma_start(out=wI_sb, in_=wI[:, :, :])
            WrT_sb = singles.tile([nE, nH, nE], BF16)
            nc.gpsimd.dma_start(out=WrT_sb, in_=WrT[:, :, :])
            mbi_sb = singles.tile([1, B, nH * nV], BF16)
            nc.sync.dma_start(out=mbi_sb, in_=mbi[:, :, :])
            ones32 = singles.tile([1, 32], BF16)
            nc.vector.memset(ones32, 1.0)
            cT_sb = singles.tile([nE, B, nV], F32)
            memM_sb = singles.tile([nV, B, nE], BF16)
            qTq_sb = singles.tile([nE, B, nQ], BF16)
            for b in range(B):
                nc.sync.dma_start(out=qTq_sb[:, b, :], in_=qT[b])
                nc.sync.dma_start(out=cT_sb[:, b, :], in_=cT[b])
                nc.gpsimd.dma_start(out=memM_sb[:, b, :], in_=memM[b])
            qh_sb = singles.tile([nE, B, nQ, nH], F32)    # fc_create out ^T
            exp_sb = singles.tile([NG, nH, nV], BF16)     # exp(masked logits)
            den_sb = singles.tile([NG, nH], F32)          # softmax denominators
            rec_sb = singles.tile([NG, nH], F32)          # 1/den
            probs_sb = singles.tile([NG, nH, nV], BF16)   # normalized probs
            HeT_sb = singles.tile([nE, B, nQ, nH], BF16)  # leaky heads^T
            hm_sb = singles.tile([nE, B, QH], F32)        # 0.01*heads scratch

            # ---- fc_createheads (batched over b: rhs = [k, B*nQ]) ----
            qTq_flat = qTq_sb[:, :, :].rearrange("k b q -> k (b q)")
            for h in range(nH):
                pqc = pqc_all[:, h, :]
                nc.tensor.matmul(pqc, WcT_sb[:, h * nE : (h + 1) * nE],
                                 qTq_flat, start=True, stop=True)
                nc.vector.tensor_scalar_add(qh_sb[:, :, :, h], pqc,
                                            bC_sb[:, h : h + 1])

            # ---- main loop: broadcast-add, tanh, wide logits matmuls ----
            for b in range(B):
                for blk in range(NBLK):
                    arg = argp.tile([nE, BLK, nV], F32)
                    qsl = qh_sb[:, b, blk * QBLK : (blk + 1) * QBLK, :]
                    k = b * NBLK + blk
                    add_eng = nc.gpsimd if k in (1, 3) else nc.vector
                    add_eng.tensor_add(
                        arg,
                        qsl[:, :, :, None].broadcast_to([nE, QBLK, nH, nV]),
                        cT_sb[:, b, None, :].broadcast_to([nE, BLK, nV]),
                    )
                    t = tp.tile([nE, BLK, nV], BF16)
                    nc.scalar.activation(out=t, in_=arg, func=AF.Tanh)
                    for q4 in range(QBLK):
                        g = b * nQ + blk * QBLK + q4
                        i, r = g // 32, g % 32
                        if r == 0:
                            # inject the mask bias row: -99999*(1-m) etc.
                            nc.tensor.matmul(pls[i], ones32,
                                             mbi_sb[:, i // 2, :],
                                             start=True, stop=False)
                        nc.tensor.matmul(
                            pls[i], wI_sb[:, r, :],
                            t[:, q4 * nH : (q4 + 1) * nH, :],
                            start=False, stop=(r == 31))

            # ---- exp + denominators (straight from PSUM) ----
            for i in range(4):
                for h in range(nH):
                    nc.scalar.activation(
                        out=exp_sb[32 * i : 32 * (i + 1), h, :],
                        in_=pls[i][:, h, :], func=AF.Exp)
            nc.vector.tensor_reduce(den_sb, exp_sb,
                                    axis=mybir.AxisListType.X,
                                    op=mybir.AluOpType.add)
            nc.vector.reciprocal(rec_sb, den_sb)
            for h in range(nH):
                nc.vector.tensor_scalar_mul(probs_sb[:, h, :],
                                            exp_sb[:, h, :],
                                            rec_sb[:, h : h + 1])

            # ---- transpose probs -> ptrT[v, h, g] via DMA xbar ----
            ptrT_sb = singles.tile([nV, nH, NG], BF16)
            for h in range(nH):
                nc.sync.dma_start_transpose(ptrT_sb[:, h, :], probs_sb[:, h, :])

            # ---- heads^T: phe[e, b, qh] = memM[v, e].T @ probsT ----
            for b in range(B):
                rhs = ptrT_sb[:, :, b * nQ : (b + 1) * nQ].rearrange(
                    "v h q -> v q h")
                nc.tensor.matmul(phe[:, b, :], memM_sb[:, b, :],
                                 rhs, start=True, stop=True)

            # ---- leaky_relu into fc_reduce layout ----
            nc.vector.tensor_scalar_mul(hm_sb, phe, 0.01)
            nc.vector.tensor_max(HeT_sb, phe, hm_sb)

            # ---- fc_reduceheads (batched over b; b_reduce added host-side) ----
            for h in range(nH):
                nc.tensor.matmul(
                    po_all, HeT_sb[:, :, :, h].rearrange("e b q -> e (b q)"),
                    WrT_sb[:, h, :], start=(h == 0), stop=(h == nH - 1))
            ob = obp.tile([B * nQ, nE], F32)
            nc.vector.tensor_copy(ob, po_all)
            nc.sync.dma_start(out=outp[:, :, :].rearrange("b q e -> (b q) e"), in_=ob)

    _split_waits(nc)
    return nc


_NC_CACHE = None


def _get_nc():
    global _NC_CACHE
    if _NC_CACHE is None:
        _NC_CACHE = _build_nc()
    return _NC_CACHE


def _prep_in_maps(inputs):
    query = np.asarray(inputs["query"], np.float32)
    context = np.asarray(inputs["context"], np.float32)
    memory = np.asarray(inputs["memory"], np.float32)
    mask = np.asarray(inputs["mask"], np.float32)
    W_create = np.asarray(inputs["W_create"], np.float32)
    b_create = np.asarray(inputs["b_create"], np.float32)
    w_logit = np.asarray(inputs["w_logit"], np.float32)
    b_logit = float(np.asarray(inputs["b_logit"], np.float32))
    W_reduce = np.asarray(inputs["W_reduce"], np.float32)

    WcT = np.ascontiguousarray(W_create.T.astype(BFNP))          # [k, he]
    WrT = np.ascontiguousarray(
        W_reduce.T.reshape(nH, nE, nE).transpose(1, 0, 2).astype(BFNP))
    bC = np.ascontiguousarray(b_create.reshape(nH, nE).T)        # [e, h]
    T = float(np.asarray(inputs["temperature"], np.float32))
    wI = np.zeros((nE, 32, 32), np.float32)
    wI[:, np.arange(32), np.arange(32)] = w_logit[:, None] / T
    wI = np.ascontiguousarray(wI.astype(BFNP))                   # (w/T) (x) I_32

    in_maps = []
    for i in range(NCORES):
        bs = slice(B * i, B * (i + 1))
        m = mask[bs]                                             # [B, nV]
        mbias = np.tile(b_logit * m / T - 30000.0 * (1.0 - m), (1, nH))
        memM = memory[bs] * m[:, :, None]                        # premasked
        in_maps.append({
            "qT": np.ascontiguousarray(
                query[bs].transpose(0, 2, 1).astype(BFNP)),
            "cT": np.ascontiguousarray(context[bs].transpose(0, 2, 1)),
            "memM": np.ascontiguousarray(memM.astype(BFNP)),
            "WcT": WcT, "WrT": WrT, "bC": bC, "wI": wI,
            "mbi": np.ascontiguousarray(mbias[None].astype(BFNP)),
        })
    return in_maps


def _run(inputs, trace=False, tmpdir=None):
    nc = _get_nc()
    in_maps = _prep_in_maps(inputs)
    res = run_bass_kernel_spmd(nc, in_maps, core_ids=list(range(NCORES)),
                               trace=trace, tmpdir=tmpdir)
    out = np.concatenate([res.results[i]["out"] for i in range(NCORES)], axis=0)
    out = out + np.asarray(inputs["b_reduce"], np.float32)[None, None, :]
    return np.ascontiguousarray(out.astype(np.float32)), res


def kernel(**inputs):
    out, _ = _run(inputs, trace=False)
    return out


# revision 34
# speedup vs baseline: 1.1132x; 1.0853x over previous
"""Bass/Trainium2 kernel for nn_Attention (additive attention, dense_transformer).

Strategy: pure data-parallel over batch N=16 across 8 NeuronCores (2 batches
per core), no collectives. Per core:
  PE   fc_create      qh_sb[e, b, q, h] (bf16, bias fused in ACT copy)
  DVE  broadcast-add  arg[e, qh, v] = qh_sb[e, qh] + cT[e, v]     (the 1x floor)
  ACT  tanh           t = tanh(arg)                                (bf16)
  PE   logits         row-select matmuls: lhsT = (w/T) x I_32 column r picks the
                      PSUM partition row; rhs = t 4qh-group (512 cols); the mask
                      bias row -B*(1-m) is injected by a K=1 ones-matmul, so
                      exp(masked) underflows to exact 0 and no mask mul is needed
  ACT  exp            straight from PSUM (bf16 out)
  DVE  reduce+recip   denominators; probs = exp * rec  (pre-normalized)
  DMA  transpose      probs -> probsT via xbar, consumed via gather-AP
  PE   heads^T        phe[e, qh] = memM[v, e].T @ probsT  (mem host-premasked)
  ACT  leaky_relu     Lrelu straight from PSUM into fc_reduce layout
  PE   fc_reduce      out[q, o] (b_reduce added host-side)

Walrus supports only ONE sync-wait per compute instruction micro-op; Tile can
emit several. `_split_waits` hoists extra waits into standalone NoOps right
before the instruction. PSUM tiles are persistent with disjoint slices per
use (PSUM slot reuse makes Tile emit same-engine WAW waits). GPSIMD tensor
ops are avoided: they contend with DVE for SBUF ports (measured 2.6x both).
"""

import numpy as np
import ml_dtypes

try:
    import concourse.bass as bass
except ImportError:
    import sys
    sys.path.insert(0, "/opt/trn_rl_repo")
    import concourse.bass as bass
import concourse.mybir as mybir
import concourse.tile as tile
from concourse.bass_utils import run_bass_kernel_spmd

N, nQ, nV, nH, nE = 16, 64, 128, 4, 128
NCORES = 8
B = N // NCORES      # batches per core
QH = nQ * nH         # 256
BLK = 32             # qh per work block
NBLK = QH // BLK     # blocks per batch (8)
QBLK = BLK // nH     # q's per block (8)
NG = B * nQ          # logits groups per core (one group = 4 qh = one q) = 128
F32 = mybir.dt.float32
BF16 = mybir.dt.bfloat16
AF = mybir.ActivationFunctionType
BFNP = ml_dtypes.bfloat16

_SPLIT_ENGINES = {
    mybir.EngineType.PE,
    mybir.EngineType.DVE,
    mybir.EngineType.Activation,
    mybir.EngineType.Pool,
    mybir.EngineType.SP,
}
_NO_SPLIT_OPS = {"TriggeredCopy", "EventSemaphore", "NoOp",
                 "UnconditionalBranch", "RegisterMove", "Halt", "BranchHint"}


def _split_waits(nc):
    nid = 0
    for f in nc.m.functions:
        for blk in f.blocks:
            out = []
            for inst in blk.instructions:
                si = inst.sync_info
                if (si is not None and len(si.on_wait) > 1
                        and inst.engine in _SPLIT_ENGINES
                        and str(inst.opcode) not in _NO_SPLIT_OPS):
                    waits = list(si.on_wait)
                    for w in waits[:-1]:
                        nid += 1
                        nop = mybir.InstNoOp(name=f"I-wsplit-{nid}",
                                             ins=[], outs=[])
                        nop.engine = inst.engine
                        nop.sync_info = mybir.SyncInfo(on_wait=[w],
                                                       on_update=[])
                        out.append(nop)
                    inst.sync_info = mybir.SyncInfo(
                        on_wait=[waits[-1]], on_update=list(si.on_update))
                out.append(inst)
            blk.instructions[:] = out


def _build_nc():
    nc = bass.Bass()
    qT = nc.declare_dram_parameter("qT", [B, nE, nQ], BF16, isOutput=False)
    cT = nc.declare_dram_parameter("cT", [B, nE, nV], BF16, isOutput=False)
    memM = nc.declare_dram_parameter("memM", [B, nV, nE], BF16, isOutput=False)
    WcT = nc.declare_dram_parameter("WcT", [nE, nH * nE], BF16, isOutput=False)
    WrT = nc.declare_dram_parameter("WrT", [nE, nH, nE], BF16, isOutput=False)
    bC = nc.declare_dram_parameter("bC", [nE, nH], F32, isOutput=False)
    wI = nc.declare_dram_parameter("wI", [nE, 32, 32], BF16, isOutput=False)
    mbi = nc.declare_dram_parameter("mbi", [1, B, nH * nV], BF16, isOutput=False)
    outp = nc.declare_dram_parameter("out", [B, nQ, nE], F32, isOutput=True)

    with tile.TileContext(nc) as tc:
        with tc.tile_pool(name="singles", bufs=1) as singles, \
             tc.tile_pool(name="argp", bufs=4) as argp, \
             tc.tile_pool(name="tp", bufs=4) as tp, \
             tc.tile_pool(name="obp", bufs=2) as obp, \
             tc.tile_pool(name="psing", bufs=1, space="PSUM") as psing:

            # ---- persistent PSUM tiles (disjoint slices) ----
            pls = [psing.tile([32, nH, nV], F32, name=f"pl{i}", tag=f"pl{i}")
                   for i in range(4)]               # logits [g%32, h, v] x4
            pqc_all = psing.tile([nE, nH, B * nQ], F32)  # fc_create out
            phe = psing.tile([nE, B, QH], F32)           # heads^T
            po_all = psing.tile([B * nQ, nE], F32)       # final out

            # ---- constants / persistent SBUF tiles (DMAs spread over queues,
            #      ordered so the pipeline can start ASAP) ----
            qTq_sb = singles.tile([nE, B, nQ], BF16)
            WcT_sb = singles.tile([nE, nH * nE], BF16)
            bC_sb = singles.tile([nE, nH], F32)
            cT_sb = singles.tile([nE, B, nV], BF16)
            mbi_sb = singles.tile([1, B, nH * nV], BF16)
            wI_sb = singles.tile([nE, 32, 32], BF16)
            memM_sb = singles.tile([nV, B, nE], BF16)
            WrT_sb = singles.tile([nE, nH, nE], BF16)
            for b in range(B):
                nc.sync.dma_start(out=qTq_sb[:, b, :], in_=qT[b])
            nc.sync.dma_start(out=WcT_sb, in_=WcT[:, :])
            nc.sync.dma_start(out=bC_sb, in_=bC[:, :])
            for b in range(B):
                nc.scalar.dma_start(out=cT_sb[:, b, :], in_=cT[b])
            nc.sync.dma_start(out=mbi_sb, in_=mbi[:, :, :])
            nc.gpsimd.dma_start(out=wI_sb, in_=wI[:, :, :])
            for b in range(B):
                nc.gpsimd.dma_start(out=memM_sb[:, b, :], in_=memM[b])
            nc.scalar.dma_start(out=WrT_sb, in_=WrT[:, :, :])
            ones32 = singles.tile([1, 32], BF16)
            nc.vector.memset(ones32, 1.0)
            qh_sb = singles.tile([nE, B, nQ, nH], BF16)   # fc_create out ^T
            exp_sb = singles.tile([NG, nH, nV], BF16)     # exp(masked logits)
            den_sb = singles.tile([NG, nH], F32)          # softmax denominators
            rec_sb = singles.tile([NG, nH], F32)          # 1/den
            probs_sb = singles.tile([NG, nH, nV], BF16)   # normalized probs
            ptrT_sb = singles.tile([nV, nH, NG], BF16)    # probs^T [v, h, g]
            HeT_sb = singles.tile([nE, B, nQ, nH], BF16)  # leaky heads^T

            # ---- fc_createheads (batched over b) ----
            qTq_flat = qTq_sb[:, :, :].rearrange("k b q -> k (b q)")
            for h in range(nH):
                pqc = pqc_all[:, h, :]
                nc.tensor.matmul(pqc, WcT_sb[:, h * nE : (h + 1) * nE],
                                 qTq_flat, start=True, stop=True)
                nc.scalar.activation(out=qh_sb[:, :, :, h], in_=pqc,
                                     func=AF.Identity, bias=bC_sb[:, h : h + 1])

            def tail_batch(b):
                """softmax + heads + fc_reduce for batch b (tiles 2b, 2b+1)."""
                gsl = slice(64 * b, 64 * (b + 1))
                nc.vector.tensor_reduce(den_sb[gsl, :], exp_sb[gsl, :, :],
                                        axis=mybir.AxisListType.X,
                                        op=mybir.AluOpType.add)
                nc.vector.reciprocal(rec_sb[gsl, :], den_sb[gsl, :])
                for h in range(nH):
                    nc.vector.tensor_scalar_mul(
                        probs_sb[gsl, h, :], exp_sb[gsl, h, :],
                        rec_sb[gsl, h : h + 1])
                    nc.sync.dma_start_transpose(
                        ptrT_sb[:, h, gsl], probs_sb[gsl, h, :])
                rhs = ptrT_sb[:, :, gsl].rearrange("v h q -> v q h")
                nc.tensor.matmul(phe[:, b, :], memM_sb[:, b, :],
                                 rhs, start=True, stop=True)
                nc.scalar.activation(
                    out=HeT_sb[:, b, :, :].rearrange("e q h -> e (q h)"),
                    in_=phe[:, b, :], func=AF.Lrelu, alpha=0.01)
                osl = po_all[64 * b : 64 * (b + 1), :]
                for h in range(nH):
                    nc.tensor.matmul(
                        osl, HeT_sb[:, b, :, h],
                        WrT_sb[:, h, :], start=(h == 0), stop=(h == nH - 1))
                ob = obp.tile([nQ, nE], F32)
                nc.vector.tensor_copy(ob, osl)
                nc.sync.dma_start(out=outp[b], in_=ob)

            # ---- main loop ----
            for b in range(B):
                for blk in range(NBLK):
                    arg = argp.tile([nE, BLK, nV], BF16)
                    qsl = qh_sb[:, b, blk * QBLK : (blk + 1) * QBLK, :]
                    nc.vector.tensor_add(
                        arg,
                        qsl[:, :, :, None].broadcast_to([nE, QBLK, nH, nV]),
                        cT_sb[:, b, None, :].broadcast_to([nE, BLK, nV]),
                    )
                    t = tp.tile([nE, BLK, nV], BF16)
                    nc.scalar.activation(out=t, in_=arg, func=AF.Tanh)
                    for q4 in range(QBLK):
                        g = b * nQ + blk * QBLK + q4
                        i, r = g // 32, g % 32
                        if r == 0:
                            nc.tensor.matmul(pls[i], ones32,
                                             mbi_sb[:, i // 2, :],
                                             start=True, stop=False)
                        nc.tensor.matmul(
                            pls[i], wI_sb[:, r, :],
                            t[:, q4 * nH : (q4 + 1) * nH, :],
                            start=False, stop=(r == 31))
                    if blk % 4 == 3:
                        # tile i = 2b + blk//4 just completed -> exp it
                        i = 2 * b + blk // 4
                        for h in range(nH):
                            nc.scalar.activation(
                                out=exp_sb[32 * i : 32 * (i + 1), h, :],
                                in_=pls[i][:, h, :], func=AF.Exp)
                tail_batch(b)

    _split_waits(nc)
    return nc


_NC_CACHE = None


def _get_nc():
    global _NC_CACHE
    if _NC_CACHE is None:
        _NC_CACHE = _build_nc()
    return _NC_CACHE


def _prep_in_maps(inputs):
    query = np.asarray(inputs["query"], np.float32)
    context = np.asarray(inputs["context"], np.float32)
    memory = np.asarray(inputs["memory"], np.float32)
    mask = np.asarray(inputs["mask"], np.float32)
    W_create = np.asarray(inputs["W_create"], np.float32)
    b_create = np.asarray(inputs["b_create"], np.float32)
    w_logit = np.asarray(inputs["w_logit"], np.float32)
    b_logit = float(np.asarray(inputs["b_logit"], np.float32))
    W_reduce = np.asarray(inputs["W_reduce"], np.float32)

    WcT = np.ascontiguousarray(W_create.T.astype(BFNP))          # [k, he]
    WrT = np.ascontiguousarray(
        W_reduce.T.reshape(nH, nE, nE).transpose(1, 0, 2).astype(BFNP))
    bC = np.ascontiguousarray(b_create.reshape(nH, nE).T)        # [e, h]
    T = float(np.asarray(inputs["temperature"], np.float32))
    wI = np.zeros((nE, 32, 32), np.float32)
    wI[:, np.arange(32), np.arange(32)] = w_logit[:, None] / T
    wI = np.ascontiguousarray(wI.astype(BFNP))                   # (w/T) (x) I_32

    in_maps = []
    for i in range(NCORES):
        bs = slice(B * i, B * (i + 1))
        m = mask[bs]                                             # [B, nV]
        mbias = np.tile(b_logit * m / T - 30000.0 * (1.0 - m), (1, nH))
        memM = memory[bs] * m[:, :, None]                        # premasked
        in_maps.append({
            "qT": np.ascontiguousarray(
                query[bs].transpose(0, 2, 1).astype(BFNP)),
            "cT": np.ascontiguousarray(
                context[bs].transpose(0, 2, 1).astype(BFNP)),
            "memM": np.ascontiguousarray(memM.astype(BFNP)),
            "WcT": WcT, "WrT": WrT, "bC": bC, "wI": wI,
            "mbi": np.ascontiguousarray(mbias[None].astype(BFNP)),
        })
    return in_maps


def _run(inputs, trace=False, tmpdir=None):
    nc = _get_nc()
    in_maps = _prep_in_maps(inputs)
    res = run_bass_kernel_spmd(nc, in_maps, core_ids=list(range(NCORES)),
                               trace=trace, tmpdir=tmpdir)
    out = np.concatenate([res.results[i]["out"] for i in range(NCORES)], axis=0)
    out = out + np.asarray(inputs["b_reduce"], np.float32)[None, None, :]
    return np.ascontiguousarray(out.astype(np.float32)), res


def kernel(**inputs):
    out, _ = _run(inputs, trace=False)
    return out


# revision 36
# speedup vs baseline: 1.1464x; 1.0298x over previous
"""Bass/Trainium2 kernel for nn_Attention (additive attention, dense_transformer).

Strategy: pure data-parallel over batch N=16 across 8 NeuronCores (2 batches
per core), no collectives. Per core:
  PE   fc_create      qh_sb[e, b, q, h] (bf16, bias fused in ACT copy)
  DVE  broadcast-add  arg[e, qh, v] = qh_sb[e, qh] + cT[e, v]     (the 1x floor)
  ACT  tanh           t = tanh(arg)                                (bf16)
  PE   logits         row-select matmuls: lhsT = (w/T) x I_32 column r picks the
                      PSUM partition row; rhs = t 4qh-group (512 cols); the mask
                      bias row -B*(1-m) is injected by a K=1 ones-matmul, so
                      exp(masked) underflows to exact 0 and no mask mul is needed
  ACT  exp            straight from PSUM (bf16 out)
  DVE  reduce+recip   denominators; probs = exp * rec  (pre-normalized)
  DMA  transpose      probs -> probsT via xbar, consumed via gather-AP
  PE   heads^T        phe[e, qh] = memM[v, e].T @ probsT  (mem host-premasked)
  ACT  leaky_relu     Lrelu straight from PSUM into fc_reduce layout
  PE   fc_reduce      out[q, o] (b_reduce added host-side)

Walrus supports only ONE sync-wait per compute instruction micro-op; Tile can
emit several. `_split_waits` hoists extra waits into standalone NoOps right
before the instruction. PSUM tiles are persistent with disjoint slices per
use (PSUM slot reuse makes Tile emit same-engine WAW waits). GPSIMD tensor
ops are avoided: they contend with DVE for SBUF ports (measured 2.6x both).
"""

import numpy as np
import ml_dtypes

try:
    import concourse.bass as bass
except ImportError:
    import sys
    sys.path.insert(0, "/opt/trn_rl_repo")
    import concourse.bass as bass
import concourse.mybir as mybir
import concourse.tile as tile
from concourse.bass_utils import run_bass_kernel_spmd

N, nQ, nV, nH, nE = 16, 64, 128, 4, 128
NCORES = 8
B = N // NCORES      # batches per core
QH = nQ * nH         # 256
BLK = 32             # qh per work block
NBLK = QH // BLK     # blocks per batch (8)
QBLK = BLK // nH     # q's per block (8)
NG = B * nQ          # logits groups per core (one group = 4 qh = one q) = 128
F32 = mybir.dt.float32
BF16 = mybir.dt.bfloat16
AF = mybir.ActivationFunctionType
BFNP = ml_dtypes.bfloat16

_SPLIT_ENGINES = {
    mybir.EngineType.PE,
    mybir.EngineType.DVE,
    mybir.EngineType.Activation,
    mybir.EngineType.Pool,
    mybir.EngineType.SP,
}
_NO_SPLIT_OPS = {"TriggeredCopy", "EventSemaphore", "NoOp",
                 "UnconditionalBranch", "RegisterMove", "Halt", "BranchHint"}


def _split_waits(nc):
    nid = 0
    for f in nc.m.functions:
        for blk in f.blocks:
            out = []
            for inst in blk.instructions:
                si = inst.sync_info
                if (si is not None and len(si.on_wait) > 1
                        and inst.engine in _SPLIT_ENGINES
                        and str(inst.opcode) not in _NO_SPLIT_OPS):
                    waits = list(si.on_wait)
                    for w in waits[:-1]:
                        nid += 1
                        nop = mybir.InstNoOp(name=f"I-wsplit-{nid}",
                                             ins=[], outs=[])
                        nop.engine = inst.engine
                        nop.sync_info = mybir.SyncInfo(on_wait=[w],
                                                       on_update=[])
                        out.append(nop)
                    inst.sync_info = mybir.SyncInfo(
                        on_wait=[waits[-1]], on_update=list(si.on_update))
                out.append(inst)
            blk.instructions[:] = out


def _build_nc():
    nc = bass.Bass()
    qT = nc.declare_dram_parameter("qT", [B, nE, nQ], BF16, isOutput=False)
    cT = nc.declare_dram_parameter("cT", [B, nE, nV], BF16, isOutput=False)
    memM = nc.declare_dram_parameter("memM", [B, nV, nE], BF16, isOutput=False)
    WcT = nc.declare_dram_parameter("WcT", [nE, nH * nE], BF16, isOutput=False)
    WrT = nc.declare_dram_parameter("WrT", [nE, nH, nE], BF16, isOutput=False)
    bC = nc.declare_dram_parameter("bC", [nE, nH], F32, isOutput=False)
    wI = nc.declare_dram_parameter("wI", [nE, 32, 32], BF16, isOutput=False)
    mbi = nc.declare_dram_parameter("mbi", [1, B, nH * nV], BF16, isOutput=False)
    outp = nc.declare_dram_parameter("out", [B, nQ, nE], F32, isOutput=True)

    with tile.TileContext(nc) as tc:
        with tc.tile_pool(name="singles", bufs=1) as singles, \
             tc.tile_pool(name="argp", bufs=4) as argp, \
             tc.tile_pool(name="tp", bufs=4) as tp, \
             tc.tile_pool(name="obp", bufs=2) as obp, \
             tc.tile_pool(name="psing", bufs=1, space="PSUM") as psing:

            # ---- persistent PSUM tiles (disjoint slices) ----
            pls = [psing.tile([32, nH, nV], F32, name=f"pl{i}", tag=f"pl{i}")
                   for i in range(4)]               # logits [g%32, h, v] x4
            pqc_all = psing.tile([nE, nH, B * nQ], F32)  # fc_create out
            phe = psing.tile([nE, B, QH], F32)           # heads^T
            po_all = psing.tile([B * nQ, nE], F32)       # final out

            # ---- constants / persistent SBUF tiles (DMAs spread over queues,
            #      ordered so the pipeline can start ASAP) ----
            qTq_sb = singles.tile([nE, B, nQ], BF16)
            WcT_sb = singles.tile([nE, nH * nE], BF16)
            bC_sb = singles.tile([nE, nH], F32)
            cT_sb = singles.tile([nE, B, nV], BF16)
            mbi_sb = singles.tile([1, B, nH * nV], BF16)
            wI_sb = singles.tile([nE, 32, 32], BF16)
            memM_sb = singles.tile([nV, B, nE], BF16)
            WrT_sb = singles.tile([nE, nH, nE], BF16)
            for b in range(B):
                nc.sync.dma_start(out=qTq_sb[:, b, :], in_=qT[b])
            for h in range(nH):
                nc.sync.dma_start(out=WcT_sb[:, h * nE : (h + 1) * nE],
                                  in_=WcT[:, h * nE : (h + 1) * nE])
            nc.sync.dma_start(out=bC_sb, in_=bC[:, :])
            for b in range(B):
                nc.scalar.dma_start(out=cT_sb[:, b, :], in_=cT[b])
            nc.sync.dma_start(out=mbi_sb, in_=mbi[:, :, :])
            nc.gpsimd.dma_start(out=wI_sb, in_=wI[:, :, :])
            for b in range(B):
                nc.gpsimd.dma_start(out=memM_sb[:, b, :], in_=memM[b])
            nc.scalar.dma_start(out=WrT_sb, in_=WrT[:, :, :])
            ones32 = singles.tile([1, 32], BF16)
            nc.vector.memset(ones32, 1.0)
            qh_sb = singles.tile([nE, B, nQ, nH], BF16)   # fc_create out ^T
            exp_sb = singles.tile([NG, nH, nV], BF16)     # exp(masked logits)
            den_sb = singles.tile([NG, nH], F32)          # softmax denominators
            rec_sb = singles.tile([NG, nH], F32)          # 1/den
            probs_sb = singles.tile([NG, nH, nV], BF16)   # normalized probs
            ptrT_sb = singles.tile([nV, nH, NG], BF16)    # probs^T [v, h, g]
            HeT_sb = singles.tile([nE, B, nQ, nH], BF16)  # leaky heads^T
            crep_sb = singles.tile([nE, B, BLK, nV], BF16)  # c replicated x BLK
            for b in range(B):
                nc.vector.tensor_copy(
                    crep_sb[:, b, :, :],
                    cT_sb[:, b, None, :].broadcast_to([nE, BLK, nV]))

            # ---- fc_createheads (batched over b) ----
            qTq_flat = qTq_sb[:, :, :].rearrange("k b q -> k (b q)")
            for h in range(nH):
                pqc = pqc_all[:, h, :]
                nc.tensor.matmul(pqc, WcT_sb[:, h * nE : (h + 1) * nE],
                                 qTq_flat, start=True, stop=True)
                nc.scalar.activation(out=qh_sb[:, :, :, h], in_=pqc,
                                     func=AF.Identity, bias=bC_sb[:, h : h + 1])

            def tail_batch(b):
                """softmax + heads + fc_reduce for batch b (tiles 2b, 2b+1)."""
                gsl = slice(64 * b, 64 * (b + 1))
                nc.vector.tensor_reduce(den_sb[gsl, :], exp_sb[gsl, :, :],
                                        axis=mybir.AxisListType.X,
                                        op=mybir.AluOpType.add)
                nc.vector.reciprocal(rec_sb[gsl, :], den_sb[gsl, :])
                for h in range(nH):
                    nc.vector.tensor_scalar_mul(
                        probs_sb[gsl, h, :], exp_sb[gsl, h, :],
                        rec_sb[gsl, h : h + 1])
                    teng = (nc.sync, nc.scalar, nc.sync, nc.scalar)[h]
                    teng.dma_start_transpose(
                        ptrT_sb[:, h, gsl], probs_sb[gsl, h, :])
                rhs = ptrT_sb[:, :, gsl].rearrange("v h q -> v q h")
                nc.tensor.matmul(phe[:, b, :], memM_sb[:, b, :],
                                 rhs, start=True, stop=True)
                nc.scalar.activation(
                    out=HeT_sb[:, b, :, :].rearrange("e q h -> e (q h)"),
                    in_=phe[:, b, :], func=AF.Lrelu, alpha=0.01)
                osl = po_all[64 * b : 64 * (b + 1), :]
                for h in range(nH):
                    nc.tensor.matmul(
                        osl, HeT_sb[:, b, :, h],
                        WrT_sb[:, h, :], start=(h == 0), stop=(h == nH - 1))
                ob = obp.tile([nQ, nE], F32)
                nc.vector.tensor_copy(ob, osl)
                nc.sync.dma_start(out=outp[b], in_=ob)

            # ---- main loop ----
            for b in range(B):
                for blk in range(NBLK):
                    arg = argp.tile([nE, BLK, nV], BF16)
                    qsl = qh_sb[:, b, blk * QBLK : (blk + 1) * QBLK, :]
                    nc.vector.tensor_add(
                        arg,
                        crep_sb[:, b, :, :],
                        qsl[:, :, :, None].broadcast_to([nE, QBLK, nH, nV]),
                    )
                    t = tp.tile([nE, BLK, nV], BF16)
                    nc.scalar.activation(out=t, in_=arg, func=AF.Tanh)
                    for q4 in range(QBLK):
                        g = b * nQ + blk * QBLK + q4
                        i, r = g // 32, g % 32
                        if r == 0:
                            nc.tensor.matmul(pls[i], ones32,
                                             mbi_sb[:, i // 2, :],
                                             start=True, stop=False)
                        nc.tensor.matmul(
                            pls[i], wI_sb[:, r, :],
                            t[:, q4 * nH : (q4 + 1) * nH, :],
                            start=False, stop=(r == 31))
                    if blk % 4 == 3:
                        # tile i = 2b + blk//4 just completed -> exp it
                        i = 2 * b + blk // 4
                        for h in range(nH):
                            nc.scalar.activation(
                                out=exp_sb[32 * i : 32 * (i + 1), h, :],
                                in_=pls[i][:, h, :], func=AF.Exp)
                tail_batch(b)

    _split_waits(nc)
    return nc


_NC_CACHE = None


def _get_nc():
    global _NC_CACHE
    if _NC_CACHE is None:
        _NC_CACHE = _build_nc()
    return _NC_CACHE


def _prep_in_maps(inputs):
    query = np.asarray(inputs["query"], np.float32)
    context = np.asarray(inputs["context"], np.float32)
    memory = np.asarray(inputs["memory"], np.float32)
    mask = np.asarray(inputs["mask"], np.float32)
    W_create = np.asarray(inputs["W_create"], np.float32)
    b_create = np.asarray(inputs["b_create"], np.float32)
    w_logit = np.asarray(inputs["w_logit"], np.float32)
    b_logit = float(np.asarray(inputs["b_logit"], np.float32))
    W_reduce = np.asarray(inputs["W_reduce"], np.float32)

    WcT = np.ascontiguousarray(W_create.T.astype(BFNP))          # [k, he]
    WrT = np.ascontiguousarray(
        W_reduce.T.reshape(nH, nE, nE).transpose(1, 0, 2).astype(BFNP))
    bC = np.ascontiguousarray(b_create.reshape(nH, nE).T)        # [e, h]
    T = float(np.asarray(inputs["temperature"], np.float32))
    wI = np.zeros((nE, 32, 32), np.float32)
    wI[:, np.arange(32), np.arange(32)] = w_logit[:, None] / T
    wI = np.ascontiguousarray(wI.astype(BFNP))                   # (w/T) (x) I_32

    in_maps = []
    for i in range(NCORES):
        bs = slice(B * i, B * (i + 1))
        m = mask[bs]                                             # [B, nV]
        mbias = np.tile(b_logit * m / T - 30000.0 * (1.0 - m), (1, nH))
        memM = memory[bs] * m[:, :, None]                        # premasked
        in_maps.append({
            "qT": np.ascontiguousarray(
                query[bs].transpose(0, 2, 1).astype(BFNP)),
            "cT": np.ascontiguousarray(
                context[bs].transpose(0, 2, 1).astype(BFNP)),
            "memM": np.ascontiguousarray(memM.astype(BFNP)),
            "WcT": WcT, "WrT": WrT, "bC": bC, "wI": wI,
            "mbi": np.ascontiguousarray(mbias[None].astype(BFNP)),
        })
    return in_maps


def _run(inputs, trace=False, tmpdir=None):
    nc = _get_nc()
    in_maps = _prep_in_maps(inputs)
    res = run_bass_kernel_spmd(nc, in_maps, core_ids=list(range(NCORES)),
                               trace=trace, tmpdir=tmpdir)
    out = np.concatenate([res.results[i]["out"] for i in range(NCORES)], axis=0)
    out = out + np.asarray(inputs["b_reduce"], np.float32)[None, None, :]
    return np.ascontiguousarray(out.astype(np.float32)), res


def kernel(**inputs):
    out, _ = _run(inputs, trace=False)
    return out
